# revision 33
# baseline (speedup 1.0000x reference)
"""DeformConv1D Trainium2 Bass kernel.

Problem: B=64, C=64, L=4096, OUTC=128, KS=3 deformable conv1d.

Math (derived from the reference, validated in numpy):
    offset[b,k,t] = sum_{c,j} w_off[k,c,j] * xp[b,c,t+j] + b_off[k]
    p = t + k + offset[b,k,t]
    fl = round_half_even(p - 0.5)   (== floor(p) except at integer p, where coef=0)
    f  = p - fl
    coef = (1 <= p <= 4096) * f * (1 - f)
    out[b,o,t] = sum_{k,c} w_conv[o,c,k] * coef[b,t,k] * xs2[b,c,fl[b,t,k]]
where xp = zero-pad(x, 1) (length 4098) and xs2[u] = xp[u] + xp[u+1].
All masked / clipped / out-of-range cases collapse to coef == 0.

Device mapping (8 NeuronCores, data-parallel over batch, 8 samples/core).
Per core, per sample:
  - time axis folded in two halves across the 128 SBUF partitions:
    partition (64h + c); half h covers t_g in [2048h, 2048h + 2048) with
    xs2 stored locally as u_l = u_g - 2032*h, u_l in [0, 2065).
  - offset conv + main conv are float32r matmuls; the two halves use
    disjoint PE row groups (tile_position (0,0) vs (64,0)) and overlap.
  - per-(t,k) coef/idx elementwise chain on DVE in a [128, 96] layout:
    partition P = 64h + m, free = 32k + i, holding t_l = 32m + i
    (i = 16u + rr, u in {0,1}, rr in [0,16)).
  - gather runs on GPSIMD ap_gather with per-16-partition-core index
    lists in order q = 2048k + 1024u + 16m + rr; the wrapped index tile
    is built via one 16-bit XBAR DMA-transpose + 12 small SBUF DMAs.
  - coef is replicated over the 64 channel partitions by 0-stride DMAs
    (fp16), one DVE multiply scales the gathered data (in q order).
  - main conv consumes contiguous q-slices; the PSUM->SBUF copies
    un-permute q -> t so the output DMA is fully contiguous.
"""

import sys

import numpy as np

sys.path.insert(0, "/opt/trn_rl_repo")

import concourse.bass as bass
import concourse.bacc as bacc
import concourse.mybir as mybir
import concourse.tile as tile
from concourse.alu_op_type import AluOpType

N_CORES = 8
B, C, L, O, KS = 64, 64, 4096, 128, 3
BPC = B // N_CORES          # samples per core
TL = 2048                   # t_local per half
H1OFF = 2032                # xs2/xp global offset of half 1
NE = 2065                   # xs2 elements per half (local u in [0, NE))
NIDX = KS * TL              # 6144 gather indices per core-list
MAGIC = 12582912.0          # 1.5 * 2^23
F32 = mybir.dt.float32
F32R = mybir.dt.float32r
F16 = mybir.dt.float16
I16 = mybir.dt.int16

_PROGRAM = None


def _bc(ap, n):
    """Prepend a 0-stride broadcast dim of size n to an AP."""
    return bass.AP(ap.tensor, ap.offset, [[0, n]] + list(ap.ap))


def _emit_sample(nc, b, pools, consts, stage=9):
    (xpp, xs2p, gatp, crepp, smallp, outp, pso, psm) = pools
    (wo_r, wk_r, base_sb, hoff_sb, x_d, out_d, T_d, cf_d, cfq_d, list_d, wrD) = consts
    v = nc.vector

    # ---- 1. load folded padded input -------------------------------------
    # xp_fold[64h+c, u_l] = xp[c, u_l + H1OFF*h];  xp = [0, x[0..4095], 0]
    xp = xpp.tile([128, NE + 1], F32, name=f"xp{b}", tag="xp")
    v.memset(xp[0:64, 0:1], 0.0)
    v.memset(xp[64:128, NE : NE + 1], 0.0)
    nc.sync.dma_start(xp[0:64, 1 : NE + 1], x_d[b, :, 0:NE])
    nc.sync.dma_start(xp[64:128, 0:NE], x_d[b, :, H1OFF - 1 : H1OFF - 1 + NE])

    # ---- 2. xs2 ----------------------------------------------------------
    xs2 = xs2p.tile([128, NE], F32, name=f"xs2{b}", tag="xs2")
    v.tensor_add(xs2[:], xp[:, 0:NE], xp[:, 1 : NE + 1])

    if stage <= 1:
        nc.sync.dma_start(out_d[b, :, 0:2048], xs2[:, 0:2048].bitcast(F32))
        return

    # ---- 3. offset conv (float32r matmuls, halves on disjoint row groups)
    # fp32r matmul operands must be produced as float32r (walrus verifier):
    # one DVE convert pass of the padded input.
    xpr = xpp.tile([128, NE + 1], F32R, name=f"xpr{b}", tag="xpr")
    v.tensor_copy(xpr[:], xp[:])
    # offmA[kk, t_g] = offset[kk, t_g]
    offmA = smallp.tile([KS, L], F32, name=f"offmA{b}", tag="offmA", bufs=1)
    for tt in range(4):
        for h in range(2):
            ps = pso.tile([KS, 512], F32, name=f"psoff{b}_{tt}_{h}", tag=f"psoff{h}")
            pl = 64 * h
            cb = tt * 512 + (16 if h else 0)
            for j in range(KS):
                nc.tensor.matmul(
                    ps[:],
                    wo_r[pl : pl + 64, j, :],
                    xpr[pl : pl + 64, cb + j : cb + j + 512],
                    start=(j == 0),
                    stop=(j == KS - 1),
                    tile_position=(pl, 0),
                )
            tg = 2048 * h + tt * 512
            if tt % 2 == 0:
                nc.scalar.copy(offmA[:, tg : tg + 512], ps[:])
            else:
                v.tensor_copy(offmA[:, tg : tg + 512], ps[:])

    # ---- 4. shuffle offsets into [128, 96]: offP[64h+m, 32k+i] = off[k, t_g]
    offP = smallp.tile([128, 96], F32, name=f"offP{b}", tag="offP")
    for k in range(KS):
        nc.scalar.dma_start(
            offP[:, 32 * k : 32 * k + 32],
            offmA[k : k + 1, :].rearrange("o (p i) -> o p i", p=128, i=32),
        )

    if stage <= 2:
        nc.sync.dma_start(out_d[b, :, 0:96], offP[:])
        return

    # ---- 5. elementwise chain -> coef (f16) and idx (i16, padded to 128)
    p_t = smallp.tile([128, 96], F32, name=f"p{b}", tag="p")
    fl_t = smallp.tile([128, 96], F32, name=f"fl{b}", tag="fl")
    f_t = smallp.tile([128, 96], F32, name=f"f{b}", tag="f")
    m_t = smallp.tile([128, 96], F32, name=f"m{b}", tag="m")
    coef_t = smallp.tile([128, 96], F32, name=f"coef{b}", tag="coef")
    cf16 = smallp.tile([128, 96], F16, name=f"cf16{b}", tag="cf16")
    u1_t = smallp.tile([128, 96], F32, name=f"u1{b}", tag="u1")
    idx_t = smallp.tile([128, 128], I16, name=f"idx{b}", tag="idx")

    v.tensor_add(p_t[:], offP[:], base_sb[:])
    v.tensor_scalar(fl_t[:], p_t[:], 0.5, MAGIC, AluOpType.subtract, AluOpType.add)
    v.tensor_scalar(fl_t[:], fl_t[:], MAGIC, None, AluOpType.subtract)
    v.tensor_sub(f_t[:], p_t[:], fl_t[:])
    # t1 = (f - 1) * f = -f(1-f); two fused mask multiplies; negate in cast
    v.scalar_tensor_tensor(
        coef_t[:], f_t[:], 1.0, f_t[:], AluOpType.subtract, AluOpType.mult
    )
    v.scalar_tensor_tensor(
        coef_t[:], p_t[:], 1.0, coef_t[:], AluOpType.is_ge, AluOpType.mult
    )
    v.scalar_tensor_tensor(
        coef_t[:], p_t[:], 4096.0, coef_t[:], AluOpType.is_le, AluOpType.mult
    )
    v.tensor_scalar(cf16[:], coef_t[:], -1.0, None, AluOpType.mult)
    # idx_local = clip(fl - 2032*(h==1), 0, NE-1) as int16
    v.tensor_scalar(
        u1_t[:], fl_t[:], hoff_sb[:], 0.0, AluOpType.subtract, AluOpType.max
    )
    v.tensor_scalar(idx_t[:, 0:96], u1_t[:], float(NE - 1), None, AluOpType.min)
    # pad cols [96:128) (feeds unused transpose rows); sourced from u1 so the
    # write has a real dependency and cannot be hoisted across slot reuse.
    v.tensor_scalar(idx_t[:, 96:128], u1_t[:, 0:32], 0.0, None, AluOpType.mult)

    # ---- 6. wrapped gather index list ------------------------------------
    # T = idx_t transposed (16-bit XBAR): T[32k+16u+rr, 64h+m] = idx
    T = smallp.tile([128, 128], I16, name=f"T{b}", tag="T")
    nc.sync.dma_start(T[:], idx_t[:], transpose=True)
    nc.sync.dma_start(T_d[b], T[:])
    # list_d[b, h, rr, 128k+64u+m] = T[32k+16u+rr, 64h+m]  (per-core list,
    # unreplicated). DRAM->DRAM: SBUF partition-split APs are mis-tracked.
    for h in range(2):
        for k in range(KS):
            s3 = T_d[b].rearrange(
                "(k2 u r) m -> k2 u r m", k2=4, u=2, r=16
            )[k, :, :, 64 * h : 64 * h + 64]
            d3 = list_d[b, h].rearrange(
                "r (k2 u m) -> k2 u r m", k2=KS, u=2, m=64
            )[k]
            (nc.sync if (h + k) % 2 else nc.scalar).dma_start(d3, s3)
    # replicate the per-core list across the 4 cores of each half (in DRAM:
    # SBUF-side partition-split write APs are mis-tracked by the dep
    # machinery), then load as one plain 2D DMA.
    for h in range(2):
        nc.sync.dma_start(
            wrD[b, 64 * h : 64 * h + 64, :].rearrange("(j r) s -> j r s", j=4, r=16),
            _bc(list_d[b, h], 4),
        )
    wr = smallp.tile([128, NIDX // 16], I16, name=f"wr{b}", tag="wr")
    nc.sync.dma_start(wr[:], wrD[b])

    # ---- 7. replicated coef: crep[64h+c, 2048k+1024u+16m+rr] = coef ------
    nc.sync.dma_start(cf_d[b], cf16[:])
    # cfq_d[b, h, q] = coef(k, t_l(m,u,rr), h),  q = 2048k + 1024u + 16m + rr
    for h in range(2):
        for u in range(2):
            s3 = cf_d[b, 64 * h : 64 * h + 64, :].rearrange(
                "m (k u2 r) -> u2 k m r", k=KS, u2=2, r=16
            )[u]
            d3 = cfq_d[b, h].rearrange(
                "(k u2 m r) -> u2 k m r", k=KS, u2=2, m=64, r=16
            )[u]
            (nc.sync if (h + u) % 2 else nc.scalar).dma_start(d3, s3)
    crep = crepp.tile([128, NIDX], F16, name=f"crep{b}", tag="crep")
    for h in range(2):
        nc.scalar.dma_start(crep[64 * h : 64 * h + 64, :], _bc(cfq_d[b, h], 64))

    if stage <= 3:
        nc.sync.dma_start(out_d[b, :, 0:96], u1_t[:])
        return

    # ---- 8. gather on GPSIMD ---------------------------------------------
    G = gatp.tile([128, NIDX], F32, name=f"G{b}", tag="G", bufs=1)
    nc.gpsimd.ap_gather(
        G[:, :, None],
        xs2[:, :, None],
        wr[:],
        channels=128,
        num_elems=NE,
        d=1,
        num_idxs=NIDX,
    )

    # ---- 9. scale ---------------------------------------------------------
    # separate F32R output tile: the walrus verifier requires every writer
    # of a fp32r-matmul operand's memory to produce float32r, so the raw
    # gather output (plain f32 -- f32r input wedges the gather ucode) and the
    # rounded scaled values must live in different tiles.
    G2 = gatp.tile([128, NIDX], F32R, name=f"G2{b}", tag="G2", bufs=2)
    v.tensor_mul(G2[:], G[:], crep[:])

    if stage <= 4:
        nc.sync.dma_start(out_d[b, :, 0:4096], G2[:, 0:4096].bitcast(F32))
        return

    # ---- 10. main conv (q-ordered psum), un-permute in copies, DMA out ---
    for h in range(2):
        pl = 64 * h
        for pair in range(2):           # pair a: q-tiles tt'=a and a+2
            pms = []
            for uu in range(2):
                tt = 2 * uu + pair
                pm = psm.tile(
                    [O, 512], F32, name=f"psm{b}_{h}_{tt}", tag=f"psm{h}", bufs=2
                )
                for k in range(KS):
                    qb = 2048 * k + 512 * tt
                    nc.tensor.matmul(
                        pm[:],
                        wk_r[pl : pl + 64, k, :],
                        G2[pl : pl + 64, qb : qb + 512],
                        start=(k == 0),
                        stop=(k == KS - 1),
                        tile_position=(pl, 0),
                    )
                pms.append(pm)
            osb = outp.tile([O, 1024], F32, name=f"osb{b}_{h}_{pair}", tag=f"osb{h}")
            for half in range(2):       # output t-tile c = 2*pair + half
                for uu in range(2):
                    # psum col n = 256*half + 16*m' + rr
                    #   -> osb col 512*half + 32*m' + 16*uu + rr
                    src = pms[uu][:, 256 * half : 256 * half + 256].rearrange(
                        "o (m r) -> o m r", m=16, r=16
                    )
                    dst = osb[:, 512 * half : 512 * half + 512].rearrange(
                        "o (m w r) -> o m w r", m=16, w=2, r=16
                    )[:, :, uu]
                    if (half + uu) % 2 == 0:
                        v.tensor_copy(dst, src)
                    else:
                        nc.scalar.copy(dst, src)
            tg = 2048 * h + 1024 * pair
            nc.sync.dma_start(out_d[b, :, tg : tg + 1024], osb[:])


def build_program(stage=9, repeat=1):
    nc = bacc.Bacc("TRN2", target_bir_lowering=False, debug=False)
    x_d = nc.dram_tensor("x8", [BPC, C, L], F32, kind="ExternalInput").ap()
    wo_d = nc.dram_tensor("wo_dup", [128, KS, KS], F32, kind="ExternalInput").ap()
    wk_d = nc.dram_tensor("wk_dup", [128, KS, O], F32, kind="ExternalInput").ap()
    base_d = nc.dram_tensor("base_c", [128, 96], F32, kind="ExternalInput").ap()
    hoff_d = nc.dram_tensor("hoff_c", [128, 1], F32, kind="ExternalInput").ap()
    out_d = nc.dram_tensor("out8", [BPC, O, L], F32, kind="ExternalOutput").ap()
    T_d = nc.dram_tensor("T_d", [BPC, 128, 128], I16, kind="Internal").ap()
    cf_d = nc.dram_tensor("cf_d", [BPC, 128, 96], F16, kind="Internal").ap()
    cfq_d = nc.dram_tensor("cfq_d", [BPC, 2, NIDX], F16, kind="Internal").ap()
    list_d = nc.dram_tensor("list_d", [BPC, 2, 16, NIDX // 16], I16, kind="Internal").ap()
    wrD = nc.dram_tensor("wrD", [BPC, 128, NIDX // 16], I16, kind="Internal").ap()

    with tile.TileContext(nc) as t:
        with (
            t.tile_pool(name="const", bufs=1) as constp,
            t.tile_pool(name="xp", bufs=2) as xpp,
            t.tile_pool(name="xs2", bufs=2) as xs2p,
            t.tile_pool(name="gat", bufs=2) as gatp,
            t.tile_pool(name="crep", bufs=2) as crepp,
            t.tile_pool(name="small", bufs=2) as smallp,
            t.tile_pool(name="outsb", bufs=2) as outp,
            t.tile_pool(name="psum_off", bufs=2, space="PSUM") as pso,
            t.tile_pool(name="psum_main", bufs=2, space="PSUM") as psm,
        ):
            wo_sb = constp.tile([128, KS, KS], F32, name="wo_sb")
            wk_sb = constp.tile([128, KS, O], F32, name="wk_sb")
            base_sb = constp.tile([128, 96], F32, name="base_sb")
            hoff_sb = constp.tile([128, 1], F32, name="hoff_sb")
            nc.sync.dma_start(wo_sb[:], wo_d)
            nc.sync.dma_start(wk_sb[:], wk_d)
            nc.sync.dma_start(base_sb[:], base_d)
            nc.sync.dma_start(hoff_sb[:], hoff_d)
            wo_r = constp.tile([128, KS, KS], F32R, name="wo_r")
            wk_r = constp.tile([128, KS, O], F32R, name="wk_r")
            nc.vector.tensor_copy(wo_r[:], wo_sb[:])
            nc.vector.tensor_copy(wk_r[:], wk_sb[:])
            pools = (xpp, xs2p, gatp, crepp, smallp, outp, pso, psm)
            consts = (
                wo_r, wk_r, base_sb, hoff_sb, x_d, out_d,
                T_d, cf_d, cfq_d, list_d, wrD,
            )
            for _r in range(repeat):
                for b in range(BPC):
                    _emit_sample(nc, b, pools, consts, stage=stage)
    nc.compile()
    return nc


def get_program():
    global _PROGRAM
    if _PROGRAM is None:
        _PROGRAM = build_program()
    return _PROGRAM


def host_inputs(x, w_off, b_off, w_conv):
    """Pure layout prep of the (runtime) inputs -> per-core in_maps."""
    x = np.ascontiguousarray(np.asarray(x, dtype=np.float32))
    w_off = np.asarray(w_off, dtype=np.float32)
    b_off = np.asarray(b_off, dtype=np.float32)
    w_conv = np.asarray(w_conv, dtype=np.float32)

    wo_half = np.transpose(w_off, (1, 2, 0))          # [c, j, k]
    wo_dup = np.ascontiguousarray(np.concatenate([wo_half, wo_half], axis=0))
    wk_half = np.transpose(w_conv, (1, 2, 0))         # [c, k, o]
    wk_dup = np.ascontiguousarray(np.concatenate([wk_half, wk_half], axis=0))

    p = np.arange(128)
    i = np.arange(32)
    k = np.arange(KS)
    t_g = (32 * p)[:, None, None] + i[None, None, :]                  # [128,1,32]
    base = t_g + k[None, :, None] + b_off[None, :, None]              # [128,3,32]
    base_c = np.ascontiguousarray(base.reshape(128, 96).astype(np.float32))
    hoff_c = np.where(p >= 64, float(H1OFF), 0.0).astype(np.float32)[:, None]
    hoff_c = np.ascontiguousarray(hoff_c)

    in_maps = []
    for core in range(N_CORES):
        in_maps.append(
            {
                "x8": x[core * BPC : (core + 1) * BPC],
                "wo_dup": wo_dup,
                "wk_dup": wk_dup,
                "base_c": base_c,
                "hoff_c": hoff_c,
            }
        )
    return in_maps


def kernel(x, w_off, b_off, w_conv):
    from concourse import bass_utils

    nc = get_program()
    in_maps = host_inputs(x, w_off, b_off, w_conv)
    res = bass_utils.run_bass_kernel_spmd(
        nc, in_maps, core_ids=list(range(N_CORES))
    )
    out = np.concatenate([r["out8"] for r in res.results], axis=0)
    return out.astype(np.float32)
